# revision 36
# baseline (speedup 1.0000x reference)
"""BERT self-attention (B=4, S=1024, D=1024, H=16) on 8 TRN2 NeuronCores.

Sharding: tensor-parallel over heads. Core c owns output dims
[c*128, (c+1)*128) of Wq/Wk/Wv (= heads 2c and 2c+1) and computes those
heads' attention for all 4 batches. seq is replicated (each core needs all
tokens). The host pre-transposes seq -> seqT [D, B*S] and the weight
shards -> [D, 128] (both cast to fp16) so every on-chip matmul operand
already has the contraction dim on partitions; all matmuls run in fp16
(11-bit mantissa, like tf32) with fp32 PSUM accumulation.

Per-core pipeline (per batch):
  qT/kT/vT [128, S] = W_shard @ seqT_b        (K=1024, N=512 chunks)
  v = PE-transpose(vT) into ones-augmented tiles [v_h0 | 1 | v_h1 | 1]
  per head h (64 dims), scores for both heads interleaved so the K=64
  matmuls pack into disjoint PE row groups:
    scoresT[j,i] = k_j . q_i   ->  expT = exp(0.125*scoresT)   (ACT)
    outT[(d,den), i] = [v_h | 1]^T @ expT     (K=1024 accumulation)
    row 64 is the softmax denominator; divide via a K=1 broadcast
    matmul of 1/den and a DVE multiply, store outT d-major.
The host transposes the gathered [head, d, token] result back to
[token, d] (layout only - all FLOPs happen on-device).

The softmax skips the max-subtraction: scores ~ N(0,1) here so exp() is
comfortably in fp32 range, and exp(x)/sum(exp(x)) is algebraically
identical to the max-shifted form.
"""

import numpy as np
from contextlib import ExitStack

import concourse.bass as bass
import concourse.tile as tile
from concourse import bacc, mybir
from concourse.bass_utils import run_bass_kernel_spmd

N_CORES = 8
B, S, D = 4, 1024, 1024
DPC = 128  # output dims per core (2 heads x 64)
HPC = 2  # heads per core
DV = 64  # head dim
KT = D // 128  # contraction tiles
NCH = S // 512  # 512-wide free-dim chunks per batch
F32 = mybir.dt.float32
F16 = mybir.dt.float16
EXP = mybir.ActivationFunctionType.Exp

# test.py may flip these to profile; the grading path leaves them alone.
TRACE = False
TRACE_KWARGS = {}
LAST_RESULTS = None

_CACHE = {}


def _emit(ctx, tc, seqT, wT, bias, ident, outcT):
    nc = tc.nc

    singles = ctx.enter_context(tc.tile_pool(name="singles", bufs=1))
    seq_pool = ctx.enter_context(tc.tile_pool(name="seq", bufs=2))
    qkv_pool = ctx.enter_context(tc.tile_pool(name="qkv", bufs=2))
    exp_pool = ctx.enter_context(tc.tile_pool(name="expT", bufs=66))
    small_pool = ctx.enter_context(tc.tile_pool(name="small", bufs=6))
    out_pool = ctx.enter_context(tc.tile_pool(name="out", bufs=4))
    psum_mm = ctx.enter_context(tc.tile_pool(name="psum_mm", bufs=5, space="PSUM"))
    psum_sc = psum_mm
    psum_pv = ctx.enter_context(tc.tile_pool(name="psum_pv", bufs=2, space="PSUM"))
    psum_tr = ctx.enter_context(tc.tile_pool(name="psum_tr", bufs=1, space="PSUM"))

    w_sb = {}
    b_sb = {}

    def load_w(name, engine=None):
        # one DMA per weight: DRAM [D, 128] -> SBUF [128, KT, 128]
        eng = engine if engine is not None else nc.sync
        wt = singles.tile([128, KT, 128], F16, tag=f"w{name}", name=f"w{name}_sb")
        eng.dma_start(wt[:], wT[name].rearrange("(k p) m -> p k m", p=128))
        w_sb[name] = wt
        bt = singles.tile([128, 1], F32, tag=f"b{name}", name=f"b{name}_sb")
        nc.gpsimd.dma_start(bt[:], bias[name][:])
        b_sb[name] = bt

    # W(q) goes over the gpsimd queue in parallel with batch 0's first
    # token quarter on the sync queue - the first QKV matmul needs both.
    load_w("q", engine=nc.gpsimd)
    id_sb = singles.tile([128, 128], F16, tag="ident", name="id_sb")
    nc.gpsimd.dma_start(id_sb[:], ident[:])
    ones_sb = singles.tile([1, DV], F16, tag="ones", name="ones_sb")
    nc.gpsimd.memset(ones_sb[:], 1.0)

    # Persistent v tiles ([v_h0 | 1 | v_h1 | 1] per 128-token block), three
    # rotating sets (vtr(b+1) and pv(b-1) are in flight simultaneously);
    # ones columns memset once.
    va_sets = []
    for s in range(3):
        tiles = []
        for t8 in range(KT):
            va = singles.tile([128, 2 * (DV + 1)], F16,
                              tag=f"vaug_{s}_{t8}", name=f"vaug_{s}_{t8}")
            nc.gpsimd.memset(va[:, DV : DV + 1], 1.0)
            nc.gpsimd.memset(va[:, 2 * DV + 1 : 2 * DV + 2], 1.0)
            tiles.append(va)
        va_sets.append(tiles)

    all_exp = {}
    qkvT_by_b = {}

    def alloc_seq(b):
        # 4 sub-tiles of 2 k-tiles each so the first QKV matmuls only wait
        # on a quarter of the batch's tokens
        return [
            seq_pool.tile([128, 2, S], F16, tag=f"seqT{j}", name=f"seqT_b{b}p{j}")
            for j in range(4)
        ]

    def emit_dma_part(b, sq, j):
        nc.sync.dma_start(
            sq[j][:],
            seqT[:, b * S : (b + 1) * S].rearrange("(k p) s -> p k s", p=128)[
                :, 2 * j : 2 * j + 2, :
            ],
        )

    def emit_dma(b):
        sq = alloc_seq(b)
        for j in range(4):
            emit_dma_part(b, sq, j)
        return sq

    def qkv_units(b, sq):
        """Closures emitting the QKV projection for batch b, 2 matmuls per
        unit (chunk accumulation order preserved by list order)."""
        units = []
        qkvT_by_b[b] = {}
        for name in ("q", "k", "v"):
            dst = qkv_pool.tile([128, S], F16, tag=f"{name}T", name=f"{name}T_b{b}")
            qkvT_by_b[b][name] = dst
            for ic in range(NCH):
                ps = psum_mm.tile([128, 512], F32, tag="mm", name=f"ps_{name}{b}{ic}")

                def mm2(name=name, ic=ic, ps=ps, kk0=0):
                    for kk in (kk0, kk0 + 1):
                        nc.tensor.matmul(
                            ps[:],
                            w_sb[name][:, kk, :],
                            sq[kk // 2][:, kk % 2, ic * 512 : (ic + 1) * 512],
                            start=(kk == 0),
                            stop=(kk == KT - 1),
                        )

                for kk0 in range(0, KT, 2):
                    units.append(lambda name=name, ic=ic, ps=ps, kk0=kk0: mm2(name, ic, ps, kk0))

                def bias_add(name=name, ic=ic, ps=ps, dst=dst):
                    nc.vector.tensor_scalar_add(
                        dst[:, ic * 512 : (ic + 1) * 512], ps[:], b_sb[name][:]
                    )

                units.append(bias_add)
        return units

    def vtr_units(b):
        """v (token-major) via PE transpose of vT into the persistent tiles."""
        units = []
        vau = va_sets[b % 3]
        vT = qkvT_by_b[b]["v"]
        for t8 in range(KT):

            def tr(t8=t8, vau=vau, vT=vT):
                pt = psum_tr.tile([128, 128], F16, tag="tr", name=f"vtr_{b}{t8}")
                nc.tensor.transpose(pt[:], vT[:, t8 * 128 : (t8 + 1) * 128], id_sb[:])
                va = vau[t8]
                nc.vector.tensor_copy(va[:, 0:DV], pt[:, 0:DV])
                nc.vector.tensor_copy(va[:, DV + 1 : 2 * DV + 1], pt[:, DV : 2 * DV])

            units.append(tr)
        return units

    def pv_units(b, ics=tuple(range(NCH))):
        """p@v + softmax division for batch b. The division's PE matmul for
        group g is deferred until after group g+1's matmuls so the DVE
        reciprocal never stalls the PE FIFO."""
        units = []
        deferred = []
        groups = [(h, ic) for h in range(HPC) for ic in ics]
        for h, ic in groups:
            pv = psum_pv.tile([DV + 1, 512], F32, tag="pv", name=f"pv_{b}{h}{ic}")
            vau = va_sets[b % 3]

            def mm2(pv=pv, vau=vau, h=h, ic=ic, t80=0):
                ex = all_exp[(b, h, ic)]  # lazy: exps are emitted by now
                for t8 in (t80, t80 + 1):
                    nc.tensor.matmul(
                        pv[:],
                        vau[t8][:, h * (DV + 1) : (h + 1) * (DV + 1)],
                        ex[t8][:],
                        start=(t8 == 0),
                        stop=(t8 == KT - 1),
                    )

            for t80 in range(0, KT, 2):
                units.append(lambda pv=pv, vau=vau, h=h, ic=ic, t80=t80: mm2(pv, vau, h, ic, t80))

            def recip(pv=pv, h=h, ic=ic):
                den = small_pool.tile([1, 512], F32, tag="den", name=f"den_{b}{h}{ic}")
                nc.vector.tensor_copy(den[:], pv[DV : DV + 1, :])
                rc = small_pool.tile([1, 512], F32, tag="recip", name=f"rc_{b}{h}{ic}")
                nc.vector.reciprocal_approx_fast(rc[:], den[:])
                rc16 = small_pool.tile([1, 512], F16, tag="recip16", name=f"rc16_{b}{h}{ic}")
                nc.vector.tensor_copy(rc16[:], rc[:])
                return rc16

            rc16_box = {}

            def recip_unit(rc16_box=rc16_box, recip=recip):
                rc16_box["t"] = recip()

            def div_unit(pv=pv, h=h, ic=ic, rc16_box=rc16_box):
                bc = psum_tr.tile([DV, 512], F32, tag="tr", name=f"bc_{b}{h}{ic}")
                nc.tensor.matmul(bc[:], ones_sb[:], rc16_box["t"][:], start=True, stop=True)
                bc_sb = small_pool.tile([DV, 512], F32, tag="bcs", name=f"bcs_{b}{h}{ic}")
                nc.vector.tensor_copy(bc_sb[:], bc[:])
                of = out_pool.tile([DV, 512], F32, tag="of", name=f"of_{b}{h}{ic}")
                nc.vector.tensor_mul(of[:], pv[0:DV, :], bc_sb[:])
                nc.sync.dma_start(
                    outcT[h * DV : (h + 1) * DV, b * S + ic * 512 : b * S + (ic + 1) * 512],
                    of[:],
                )

            units.append(recip_unit)
            deferred.append(div_unit)
            if len(deferred) > 1:
                units.append(deferred.pop(0))
        units.extend(deferred)
        units.append(lambda b=b, groups=groups: [all_exp.pop((b, h, ic)) for h, ic in groups])
        return units

    def emit_scores_interleaved(b, filler):
        """Scores+exp for batch b (heads packed into disjoint PE row
        groups), with `filler` units threaded between score pairs so the
        ACT-paced PSUM recycling never idles the PE."""
        fq = list(filler)
        fi = 0
        n_pairs = NCH * KT
        pair = 0
        for ic in range(NCH):
            for t8 in range(KT):
                for h in range(HPC):
                    hs = slice(h * DV, (h + 1) * DV)
                    ps = psum_sc.tile([128, 512], F32, tag="mm", name=f"sc_{b}{h}{ic}{t8}")
                    nc.tensor.matmul(
                        ps[:],
                        qkvT_by_b[b]["k"][hs, t8 * 128 : (t8 + 1) * 128],
                        qkvT_by_b[b]["q"][hs, ic * 512 : (ic + 1) * 512],
                        start=True,
                        stop=True,
                    )
                    et = exp_pool.tile([128, 512], F16, tag="expT", name=f"ex_{b}{h}{ic}{t8}")
                    nc.scalar.activation(et[:], ps[:], EXP, scale=0.125)
                    all_exp.setdefault((b, h, ic), []).append(et)
                pair += 1
                # spread filler evenly across the remaining pairs
                want = (pair * len(fq)) // n_pairs
                while fi < want:
                    fq[fi]()
                    fi += 1
        while fi < len(fq):
            fq[fi]()
            fi += 1

    # Pipeline: period b emits scores(b) interleaved with pv(b-1), the
    # divisions of b-1, qkv(b+1) and the v-transposes of b+1 — everything
    # the PE can chew on while ACT works through batch b's exps.
    sq = alloc_seq(0)
    emit_dma_part(0, sq, 0)
    load_w("k")
    load_w("v")
    for j in range(1, 4):
        emit_dma_part(0, sq, j)
    for u in qkv_units(0, sq):
        u()
    for u in vtr_units(0):
        u()
    # Each period b: scores(b) paced by ACT, interleaved with qkv(b+1),
    # the ic1 p@v of b-1, the ic0 p@v of b itself (its exps land halfway
    # through the period; the Tile scheduler inserts the waits), and the
    # v-transposes for b+1.
    for b in range(B):
        filler = []
        if b + 1 < B:
            sq_next = emit_dma(b + 1)
            filler += qkv_units(b + 1, sq_next)
        if b > 0:
            filler += pv_units(b - 1, ics=(1,))
        filler += pv_units(b, ics=(0,))
        if b + 1 < B:
            filler += vtr_units(b + 1)
        emit_scores_interleaved(b, filler)
    for u in pv_units(B - 1, ics=(1,)):
        u()


def _build():
    if "nc" in _CACHE:
        return _CACHE["nc"]
    nc = bacc.Bacc(
        "TRN2",
        target_bir_lowering=False,
        debug=False,
        enable_asserts=False,
        num_devices=N_CORES,
    )
    seqT = nc.dram_tensor("seqT", [D, B * S], F16, kind="ExternalInput").ap()
    wT = {
        name: nc.dram_tensor(f"w{name}T", [D, DPC], F16, kind="ExternalInput").ap()
        for name in ("q", "k", "v")
    }
    bias = {
        name: nc.dram_tensor(f"b{name}", [DPC, 1], F32, kind="ExternalInput").ap()
        for name in ("q", "k", "v")
    }
    ident = nc.dram_tensor("ident", [128, 128], F16, kind="ExternalInput").ap()
    outcT = nc.dram_tensor("outcT", [HPC * DV, B * S], F32, kind="ExternalOutput").ap()

    with tile.TileContext(nc) as tc:
        with ExitStack() as ctx:
            _emit(ctx, tc, seqT, wT, bias, ident, outcT)
    nc.compile()
    _CACHE["nc"] = nc
    return nc


def make_in_maps(seq, Wq, bq, Wk, bk, Wv, bv):
    f16 = np.float16
    seqT_full = np.ascontiguousarray(seq.reshape(B * S, D).T.astype(f16))
    ident = np.eye(128, dtype=f16)
    in_maps = []
    for c in range(N_CORES):
        sl = slice(c * DPC, (c + 1) * DPC)
        in_maps.append(
            {
                "seqT": seqT_full,
                "wqT": np.ascontiguousarray(Wq[sl].T.astype(f16)),
                "wkT": np.ascontiguousarray(Wk[sl].T.astype(f16)),
                "wvT": np.ascontiguousarray(Wv[sl].T.astype(f16)),
                "bq": np.ascontiguousarray(bq[sl].reshape(DPC, 1).astype(np.float32)),
                "bk": np.ascontiguousarray(bk[sl].reshape(DPC, 1).astype(np.float32)),
                "bv": np.ascontiguousarray(bv[sl].reshape(DPC, 1).astype(np.float32)),
                "ident": ident,
            }
        )
    return in_maps


def assemble(results):
    """[cores][h*64+d, b*1024+i] -> [B, S, D]"""
    out = np.empty((B, S, D), np.float32)
    for c in range(N_CORES):
        r = results[c]["outcT"].reshape(DPC, B, S)  # [hd, b, i]
        out[:, :, c * DPC : (c + 1) * DPC] = r.transpose(1, 2, 0)
    return out


def kernel(seq, Wq, bq, Wk, bk, Wv, bv):
    global LAST_RESULTS
    nc = _build()
    in_maps = make_in_maps(seq, Wq, bq, Wk, bk, Wv, bv)
    res = run_bass_kernel_spmd(
        nc, in_maps, core_ids=list(range(N_CORES)), trace=TRACE, **TRACE_KWARGS
    )
    LAST_RESULTS = res
    return assemble(res.results)


# revision 37
# speedup vs baseline: 1.1194x; 1.1194x over previous
"""BERT self-attention (B=4, S=1024, D=1024, H=16) on 8 TRN2 NeuronCores.

Sharding: tensor-parallel over heads. Core c owns output dims
[c*128, (c+1)*128) of Wq/Wk/Wv (= heads 2c and 2c+1) and computes those
heads' attention for all 4 batches. seq is replicated (each core needs all
tokens). The host pre-transposes seq -> seqT [D, B*S] and the weight
shards -> [D, 128] (both cast to fp16) so every on-chip matmul operand
already has the contraction dim on partitions; all matmuls run in fp16
(11-bit mantissa, like tf32) with fp32 PSUM accumulation.

Per-core pipeline (per batch):
  qT/kT/vT [128, S] = W_shard @ seqT_b        (K=1024, N=512 chunks)
  v = PE-transpose(vT) into ones-augmented tiles [v_h0 | 1 | v_h1 | 1]
  per head h (64 dims), scores for both heads interleaved so the K=64
  matmuls pack into disjoint PE row groups:
    scoresT[j,i] = k_j . q_i   ->  expT = exp(0.125*scoresT)   (ACT)
    outT[(d,den), i] = [v_h | 1]^T @ expT     (K=1024 accumulation)
    row 64 is the softmax denominator; divide via a K=1 broadcast
    matmul of 1/den and a DVE multiply, store outT d-major.
The host transposes the gathered [head, d, token] result back to
[token, d] (layout only - all FLOPs happen on-device).

The softmax skips the max-subtraction: scores ~ N(0,1) here so exp() is
comfortably in fp32 range, and exp(x)/sum(exp(x)) is algebraically
identical to the max-shifted form.
"""

import numpy as np
from contextlib import ExitStack

import concourse.bass as bass
import concourse.tile as tile
from concourse import bacc, mybir
from concourse.bass_utils import run_bass_kernel_spmd

N_CORES = 8
B, S, D = 4, 1024, 1024
DPC = 128  # output dims per core (2 heads x 64)
HPC = 2  # heads per core
DV = 64  # head dim
KT = D // 128  # contraction tiles
NCH = S // 512  # 512-wide free-dim chunks per batch
F32 = mybir.dt.float32
F16 = mybir.dt.float16
EXP = mybir.ActivationFunctionType.Exp

# test.py may flip these to profile; the grading path leaves them alone.
TRACE = False
TRACE_KWARGS = {}
LAST_RESULTS = None

_CACHE = {}


def _emit(ctx, tc, seqT, wT, bias, ident, outcT):
    nc = tc.nc

    singles = ctx.enter_context(tc.tile_pool(name="singles", bufs=1))
    seq_pool = ctx.enter_context(tc.tile_pool(name="seq", bufs=2))
    qkv_pool = ctx.enter_context(tc.tile_pool(name="qkv", bufs=2))
    exp_pool = ctx.enter_context(tc.tile_pool(name="expT", bufs=66))
    small_pool = ctx.enter_context(tc.tile_pool(name="small", bufs=6))
    out_pool = ctx.enter_context(tc.tile_pool(name="out", bufs=4))
    psum_mm = ctx.enter_context(tc.tile_pool(name="psum_mm", bufs=5, space="PSUM"))
    psum_sc = psum_mm
    psum_pv = ctx.enter_context(tc.tile_pool(name="psum_pv", bufs=2, space="PSUM"))
    psum_tr = ctx.enter_context(tc.tile_pool(name="psum_tr", bufs=1, space="PSUM"))

    w_sb = {}
    b_sb = {}

    def load_w(name, engine=None):
        # one DMA per weight: DRAM [D, 128] -> SBUF [128, KT, 128]
        eng = engine if engine is not None else nc.sync
        wt = singles.tile([128, KT, 128], F16, tag=f"w{name}", name=f"w{name}_sb")
        eng.dma_start(wt[:], wT[name].rearrange("(k p) m -> p k m", p=128))
        w_sb[name] = wt
        bt = singles.tile([128, 1], F32, tag=f"b{name}", name=f"b{name}_sb")
        nc.gpsimd.dma_start(bt[:], bias[name][:])
        b_sb[name] = bt

    # W(q) goes over the gpsimd queue in parallel with batch 0's first
    # token quarter on the sync queue - the first QKV matmul needs both.
    load_w("q", engine=nc.gpsimd)
    id_sb = singles.tile([128, 128], F16, tag="ident", name="id_sb")
    nc.gpsimd.dma_start(id_sb[:], ident[:])
    ones_sb = singles.tile([1, DV], F16, tag="ones", name="ones_sb")
    nc.gpsimd.memset(ones_sb[:], 1.0)

    # Persistent v tiles ([v_h0 | 1 | v_h1 | 1] per 128-token block), three
    # rotating sets (vtr(b+1) and pv(b-1) are in flight simultaneously);
    # ones columns memset once.
    va_sets = []
    for s in range(3):
        tiles = []
        for t8 in range(KT):
            va = singles.tile([128, 2 * (DV + 1)], F16,
                              tag=f"vaug_{s}_{t8}", name=f"vaug_{s}_{t8}")
            nc.gpsimd.memset(va[:, DV : DV + 1], 1.0)
            nc.gpsimd.memset(va[:, 2 * DV + 1 : 2 * DV + 2], 1.0)
            tiles.append(va)
        va_sets.append(tiles)

    all_exp = {}
    qkvT_by_b = {}

    def alloc_seq(b):
        # 4 sub-tiles of 2 k-tiles each so the first QKV matmuls only wait
        # on a quarter of the batch's tokens
        return [
            seq_pool.tile([128, 2, S], F16, tag=f"seqT{j}", name=f"seqT_b{b}p{j}")
            for j in range(4)
        ]

    def emit_dma_part(b, sq, j):
        nc.sync.dma_start(
            sq[j][:],
            seqT[:, b * S : (b + 1) * S].rearrange("(k p) s -> p k s", p=128)[
                :, 2 * j : 2 * j + 2, :
            ],
        )

    def emit_dma(b):
        sq = alloc_seq(b)
        for j in range(4):
            emit_dma_part(b, sq, j)
        return sq

    def qkv_units(b, sq):
        """Closures emitting the QKV projection for batch b, 2 matmuls per
        unit (chunk accumulation order preserved by list order)."""
        units = []
        qkvT_by_b[b] = {}
        for name in ("q", "k", "v"):
            dst = qkv_pool.tile([128, S], F16, tag=f"{name}T", name=f"{name}T_b{b}")
            qkvT_by_b[b][name] = dst
            for ic in range(NCH):
                ps = psum_mm.tile([128, 512], F32, tag="mm", name=f"ps_{name}{b}{ic}")

                def mm2(name=name, ic=ic, ps=ps, kk0=0):
                    for kk in (kk0, kk0 + 1):
                        nc.tensor.matmul(
                            ps[:],
                            w_sb[name][:, kk, :],
                            sq[kk // 2][:, kk % 2, ic * 512 : (ic + 1) * 512],
                            start=(kk == 0),
                            stop=(kk == KT - 1),
                        )

                for kk0 in range(0, KT, 2):
                    units.append(lambda name=name, ic=ic, ps=ps, kk0=kk0: mm2(name, ic, ps, kk0))

                def bias_add(name=name, ic=ic, ps=ps, dst=dst):
                    nc.vector.tensor_scalar_add(
                        dst[:, ic * 512 : (ic + 1) * 512], ps[:], b_sb[name][:]
                    )

                units.append(bias_add)
        return units

    def vtr_units(b):
        """v (token-major) via PE transpose of vT into the persistent tiles."""
        units = []
        vau = va_sets[b % 3]
        vT = qkvT_by_b[b]["v"]
        for t8 in range(KT):

            def tr(t8=t8, vau=vau, vT=vT):
                pt = psum_tr.tile([128, 128], F16, tag="tr", name=f"vtr_{b}{t8}")
                nc.tensor.transpose(pt[:], vT[:, t8 * 128 : (t8 + 1) * 128], id_sb[:])
                va = vau[t8]
                nc.vector.tensor_copy(va[:, 0:DV], pt[:, 0:DV])
                nc.vector.tensor_copy(va[:, DV + 1 : 2 * DV + 1], pt[:, DV : 2 * DV])

            units.append(tr)
        return units

    def pv_units(b, ics=tuple(range(NCH))):
        """p@v + softmax division for batch b. The division's PE matmul for
        group g is deferred until after group g+1's matmuls so the DVE
        reciprocal never stalls the PE FIFO."""
        units = []
        deferred = []
        groups = [(h, ic) for h in range(HPC) for ic in ics]
        for h, ic in groups:
            pv = psum_pv.tile([DV + 1, 512], F32, tag="pv", name=f"pv_{b}{h}{ic}")
            vau = va_sets[b % 3]

            def mm2(pv=pv, vau=vau, h=h, ic=ic, t80=0):
                ex = all_exp[(b, h, ic)]  # lazy: exps are emitted by now
                for t8 in (t80, t80 + 1):
                    nc.tensor.matmul(
                        pv[:],
                        vau[t8][:, h * (DV + 1) : (h + 1) * (DV + 1)],
                        ex[t8][:],
                        start=(t8 == 0),
                        stop=(t8 == KT - 1),
                    )

            for t80 in range(0, KT, 2):
                units.append(lambda pv=pv, vau=vau, h=h, ic=ic, t80=t80: mm2(pv, vau, h, ic, t80))

            def recip(pv=pv, h=h, ic=ic):
                den = small_pool.tile([1, 512], F32, tag="den", name=f"den_{b}{h}{ic}")
                nc.vector.tensor_copy(den[:], pv[DV : DV + 1, :])
                rc = small_pool.tile([1, 512], F32, tag="recip", name=f"rc_{b}{h}{ic}")
                nc.vector.reciprocal_approx_fast(rc[:], den[:])
                rc16 = small_pool.tile([1, 512], F16, tag="recip16", name=f"rc16_{b}{h}{ic}")
                nc.vector.tensor_copy(rc16[:], rc[:])
                return rc16

            rc16_box = {}

            def recip_unit(rc16_box=rc16_box, recip=recip):
                rc16_box["t"] = recip()

            def div_unit(pv=pv, h=h, ic=ic, rc16_box=rc16_box):
                bc = psum_tr.tile([DV, 512], F32, tag="tr", name=f"bc_{b}{h}{ic}")
                nc.tensor.matmul(bc[:], ones_sb[:], rc16_box["t"][:], start=True, stop=True)
                bc_sb = small_pool.tile([DV, 512], F32, tag="bcs", name=f"bcs_{b}{h}{ic}")
                nc.vector.tensor_copy(bc_sb[:], bc[:])
                of = out_pool.tile([DV, 512], F32, tag="of", name=f"of_{b}{h}{ic}")
                nc.vector.tensor_mul(of[:], pv[0:DV, :], bc_sb[:])
                nc.sync.dma_start(
                    outcT[h * DV : (h + 1) * DV, b * S + ic * 512 : b * S + (ic + 1) * 512],
                    of[:],
                )

            units.append(recip_unit)
            deferred.append(div_unit)
            if len(deferred) > 1:
                units.append(deferred.pop(0))
        units.extend(deferred)
        units.append(lambda b=b, groups=groups: [all_exp.pop((b, h, ic)) for h, ic in groups])
        return units

    def emit_scores_interleaved(b, filler):
        """Scores+exp for batch b (heads packed into disjoint PE row
        groups), with `filler` units threaded between score pairs so the
        ACT-paced PSUM recycling never idles the PE."""
        fq = list(filler)
        fi = 0
        n_pairs = NCH * KT
        pair = 0
        for ic in range(NCH):
            for t8 in range(KT):
                for h in range(HPC):
                    hs = slice(h * DV, (h + 1) * DV)
                    ps = psum_sc.tile([128, 512], F32, tag="mm", name=f"sc_{b}{h}{ic}{t8}")
                    nc.tensor.matmul(
                        ps[:],
                        qkvT_by_b[b]["k"][hs, t8 * 128 : (t8 + 1) * 128],
                        qkvT_by_b[b]["q"][hs, ic * 512 : (ic + 1) * 512],
                        start=True,
                        stop=True,
                    )
                    et = exp_pool.tile([128, 512], F16, tag="expT", name=f"ex_{b}{h}{ic}{t8}")
                    nc.scalar.activation(et[:], ps[:], EXP, scale=0.125)
                    all_exp.setdefault((b, h, ic), []).append(et)
                pair += 1
                # spread filler evenly across the remaining pairs
                want = (pair * len(fq)) // n_pairs
                while fi < want:
                    fq[fi]()
                    fi += 1
        while fi < len(fq):
            fq[fi]()
            fi += 1

    # Pipeline: period b emits scores(b) interleaved with pv(b-1), the
    # divisions of b-1, qkv(b+1) and the v-transposes of b+1 — everything
    # the PE can chew on while ACT works through batch b's exps.
    sq = alloc_seq(0)
    emit_dma_part(0, sq, 0)
    load_w("k")
    load_w("v")
    for j in range(1, 4):
        emit_dma_part(0, sq, j)
    for u in qkv_units(0, sq):
        u()
    for u in vtr_units(0):
        u()
    # Each period b: scores(b) paced by ACT, interleaved with qkv(b+1),
    # p@v of b-1 and the v-transposes for b+1.
    for b in range(B):
        filler = []
        if b + 1 < B:
            sq_next = emit_dma(b + 1)
            filler += qkv_units(b + 1, sq_next)
        if b > 0:
            filler += pv_units(b - 1)
        if b + 1 < B:
            filler += vtr_units(b + 1)
        else:
            # last period: pull the ic0 halves of the final batch's p@v in
            # (their exps finish halfway through this period)
            filler += pv_units(b, ics=(0,))
        emit_scores_interleaved(b, filler)
    for u in pv_units(B - 1, ics=(1,)):
        u()


def _build():
    if "nc" in _CACHE:
        return _CACHE["nc"]
    nc = bacc.Bacc(
        "TRN2",
        target_bir_lowering=False,
        debug=False,
        enable_asserts=False,
        num_devices=N_CORES,
    )
    seqT = nc.dram_tensor("seqT", [D, B * S], F16, kind="ExternalInput").ap()
    wT = {
        name: nc.dram_tensor(f"w{name}T", [D, DPC], F16, kind="ExternalInput").ap()
        for name in ("q", "k", "v")
    }
    bias = {
        name: nc.dram_tensor(f"b{name}", [DPC, 1], F32, kind="ExternalInput").ap()
        for name in ("q", "k", "v")
    }
    ident = nc.dram_tensor("ident", [128, 128], F16, kind="ExternalInput").ap()
    outcT = nc.dram_tensor("outcT", [HPC * DV, B * S], F32, kind="ExternalOutput").ap()

    with tile.TileContext(nc) as tc:
        with ExitStack() as ctx:
            _emit(ctx, tc, seqT, wT, bias, ident, outcT)
    nc.compile()
    _CACHE["nc"] = nc
    return nc


def make_in_maps(seq, Wq, bq, Wk, bk, Wv, bv):
    f16 = np.float16
    seqT_full = np.ascontiguousarray(seq.reshape(B * S, D).T.astype(f16))
    ident = np.eye(128, dtype=f16)
    in_maps = []
    for c in range(N_CORES):
        sl = slice(c * DPC, (c + 1) * DPC)
        in_maps.append(
            {
                "seqT": seqT_full,
                "wqT": np.ascontiguousarray(Wq[sl].T.astype(f16)),
                "wkT": np.ascontiguousarray(Wk[sl].T.astype(f16)),
                "wvT": np.ascontiguousarray(Wv[sl].T.astype(f16)),
                "bq": np.ascontiguousarray(bq[sl].reshape(DPC, 1).astype(np.float32)),
                "bk": np.ascontiguousarray(bk[sl].reshape(DPC, 1).astype(np.float32)),
                "bv": np.ascontiguousarray(bv[sl].reshape(DPC, 1).astype(np.float32)),
                "ident": ident,
            }
        )
    return in_maps


def assemble(results):
    """[cores][h*64+d, b*1024+i] -> [B, S, D]"""
    out = np.empty((B, S, D), np.float32)
    for c in range(N_CORES):
        r = results[c]["outcT"].reshape(DPC, B, S)  # [hd, b, i]
        out[:, :, c * DPC : (c + 1) * DPC] = r.transpose(1, 2, 0)
    return out


def kernel(seq, Wq, bq, Wk, bk, Wv, bv):
    global LAST_RESULTS
    nc = _build()
    in_maps = make_in_maps(seq, Wq, bq, Wk, bk, Wv, bv)
    res = run_bass_kernel_spmd(
        nc, in_maps, core_ids=list(range(N_CORES)), trace=TRACE, **TRACE_KWARGS
    )
    LAST_RESULTS = res
    return assemble(res.results)
